# revision 17
# baseline (speedup 1.0000x reference)
"""Multi-head attention (B=4, S=2048, D=1024, H=16, d_k=64) on 8 TRN2 NeuronCores.

Sharding: batch x head-half grid. Core c handles batch c//2 and head-half c%2
(8 of 16 heads). W_q/W_k/W_v are column-split, W_o row-split (tensor parallel);
the two partial outputs per batch are summed on the host (+bo also host-side).

All matmul operands are bf16 (fp32 PSUM accumulation). Scores pairs run
2x-concurrent in the PE array via row tiling (even head at rows 0:64, odd at
64:128 - auto tile_position). Steady state is ACT-bound: one exp([128,1024])
per iteration (~1.13us) against ~0.93us of PE work, so everything else hides
behind the exp stream.

Schedule: 256 iterations (nb, hp, sk), lookahead 2 on scores, LAG 2 on PV
(body i runs exp(i), sc(i+2), fillers, pv(i-2)) so the PE consumes only
2-iteration-old probs and never stalls on ACT. Fillers (projections,
out-projections, softmax normalization) are 2-matmul micro-ops popped at a
fixed rate after sc(i+2), keeping ACT fed. The prologue warms the PE p-state
with tiny matmuls during the initial DMA wait and defers the V-projection to
fillers so the first exp fires ~18us in. The tail staggers block-3 out-proj
psum groups so the PE stays busy through the last pair's normalization, and
its output DMAs ride the sync queue so the slow gpsimd drain overlaps.
"""

from collections import deque
from contextlib import ExitStack

import numpy as np
import ml_dtypes

import concourse.bass as bass
import concourse.mybir as mybir
import concourse.tile as tile
from concourse import bacc
from concourse.bass_utils import run_bass_kernel_spmd

P = 128
S = 2048
DM = 1024          # d_model
DH = 512           # per-core projected dim (8 heads x 64)
DK = 64
NH = 8             # heads per core
NHP = 4            # head pairs per core
SQB = 512          # Sq block width
NB = S // SQB      # 4 blocks
SKT = S // P       # 16 Sk tiles
DIT = DM // P      # 8 d_in tiles
DST = DH // P      # 4 d_out 128-slices (= head pairs)
T = NB * NHP * SKT # 256 attention iterations
NWARM = 26         # p-state warm-up matmuls during the prologue DMA wait

f32 = mybir.dt.float32
bf16 = mybir.dt.bfloat16
EXP = mybir.ActivationFunctionType.Exp
ADD = mybir.AluOpType.add
BF = ml_dtypes.bfloat16


def build():
    nc = bacc.Bacc("TRN2", target_bir_lowering=False, debug=False)

    qt = nc.declare_dram_parameter("qt", [DIT, NB, P, SQB], bf16, isOutput=False)
    kt = nc.declare_dram_parameter("kt", [DIT, NB, P, SQB], bf16, isOutput=False)
    vt = nc.declare_dram_parameter("vt", [DIT, NB, P, SQB], bf16, isOutput=False)
    # ds-major weight layouts: the ds=0 quarter arrives first so the first
    # projection group can start early.
    wq = nc.declare_dram_parameter("wq", [DST, P, DIT, P], bf16, isOutput=False)
    wk = nc.declare_dram_parameter("wk", [DST, P, DIT, P], bf16, isOutput=False)
    wv = nc.declare_dram_parameter("wv", [P, DIT, DH], bf16, isOutput=False)
    wo = nc.declare_dram_parameter("wo", [P, NHP, 2, DH], bf16, isOutput=False)
    bqt = nc.declare_dram_parameter("bqt", [P, DST], f32, isOutput=False)
    bkt = nc.declare_dram_parameter("bkt", [P, DST], f32, isOutput=False)
    bv = nc.declare_dram_parameter("bv", [1, DH], f32, isOutput=False)
    out = nc.declare_dram_parameter("out", [S, DM], bf16, isOutput=True)

    scr = nc.dram_tensor("scr", [NB, NH, SQB], f32)

    with tile.TileContext(nc) as tc, ExitStack() as ctx:
        const = ctx.enter_context(tc.tile_pool(name="const", bufs=1))
        kT_pool = ctx.enter_context(tc.tile_pool(name="kT", bufs=1))
        vA_pool = ctx.enter_context(tc.tile_pool(name="vA", bufs=1))
        xin_pool = ctx.enter_context(tc.tile_pool(name="xin", bufs=10))
        wkv_pool = ctx.enter_context(tc.tile_pool(name="wkv", bufs=1))

        ps_mm = ctx.enter_context(tc.tile_pool(name="ps_mm", bufs=2, space="PSUM"))
        ps_big = ctx.enter_context(tc.tile_pool(name="ps_big", bufs=2, space="PSUM"))
        ps_attn = ctx.enter_context(tc.tile_pool(name="ps_attn", bufs=2, space="PSUM"))

        # constants first: the warm-up matmuls depend only on these memsets
        ones128 = const.tile([P, NH], bf16)
        nc.vector.memset(ones128, 1.0)
        onescolf = const.tile([P, DK], f32)
        nc.vector.memset(onescolf, 1.0)

        # ---- prologue-critical DMAs, most-urgent first ----
        # wk/wq ds=0 quarters split per-di so the first projection matmuls can
        # start after a single 32KB chunk instead of a whole 256KB quarter.
        wk_sb = wkv_pool.tile([P, DST, DIT, P], bf16)
        wq_sb = const.tile([P, DST, DIT, P], bf16)
        for di in range(DIT):
            e = nc.sync if di % 2 == 0 else nc.gpsimd
            e.dma_start(out=wk_sb[:, 0, di, :], in_=wk[0, :, di, :])

        # x-inputs live as one [P, DIT, SQB] tile per sequence block: one DMA
        # trigger (~600ns of queue time) instead of eight, so the prologue
        # queue drains ~3x sooner. First-use tiles are split in halves for
        # startup latency. Readers slice [:, di, ...].
        def load_x_whole(src, skb, tag, bufs, eng):
            t = wkv_pool.tile([P, DIT, SQB], bf16, tag=tag, bufs=bufs,
                              name=f"{tag}{skb}")
            eng.dma_start(out=t, in_=src[:, skb].rearrange("d p q -> p d q"))
            return t

        kx0 = wkv_pool.tile([P, DIT, SQB], bf16, tag="kx", bufs=4, name="kx0")
        nc.sync.dma_start(out=kx0[:, 0:4, :],
                          in_=kt[0:4, 0].rearrange("d p q -> p d q"))
        nc.gpsimd.dma_start(out=kx0[:, 4:8, :],
                            in_=kt[4:8, 0].rearrange("d p q -> p d q"))
        kxs = {0: kx0}
        for di in range(DIT):
            e = nc.sync if di % 2 == 0 else nc.gpsimd
            e.dma_start(out=wq_sb[:, 0, di, :], in_=wq[0, :, di, :])
        qx0 = xin_pool.tile([P, DIT, SQB], bf16, tag="xin", bufs=2, name="qx0")
        nc.sync.dma_start(out=qx0[:, 0:4, :],
                          in_=qt[0:4, 0].rearrange("d p q -> p d q"))
        nc.gpsimd.dma_start(out=qx0[:, 4:8, :],
                            in_=qt[4:8, 0].rearrange("d p q -> p d q"))
        qx = {0: qx0}
        bkt_sb = const.tile([P, DST], f32)
        nc.sync.dma_start(out=bkt_sb, in_=bkt[:, :])
        bqt_sb = const.tile([P, DST], f32)
        nc.sync.dma_start(out=bqt_sb, in_=bqt[:, :])
        bv_sb = const.tile([P, DH], f32)
        nc.sync.dma_start(out=bv_sb, in_=bv[0, :].partition_broadcast(P))
        wv_sb = wkv_pool.tile([P, DIT, DH], bf16)
        nc.gpsimd.dma_start(out=wv_sb[:, 0:4, :], in_=wv[:, 0:4, :])
        nc.sync.dma_start(out=wv_sb[:, 4:DIT, :], in_=wv[:, 4:DIT, :])
        vx0 = wkv_pool.tile([P, DIT, SQB], bf16, tag="vx", bufs=2, name="vx0")
        nc.sync.dma_start(out=vx0[:, 0:4, :],
                          in_=vt[0:4, 0].rearrange("d p q -> p d q"))
        nc.gpsimd.dma_start(out=vx0[:, 4:8, :],
                            in_=vt[4:8, 0].rearrange("d p q -> p d q"))
        vxs = {0: vx0}
        nc.gpsimd.dma_start(out=wk_sb[:, 1:DST], in_=wk[1:DST].
                            rearrange("s p d q -> p s d q"))
        nc.gpsimd.dma_start(out=wq_sb[:, 1:DST], in_=wq[1:DST].
                            rearrange("s p d q -> p s d q"))
        kxs[1] = load_x_whole(kt, 1, "kx", 4, nc.sync)
        vxs[1] = load_x_whole(vt, 1, "vx", 2, nc.gpsimd)
        kxs[2] = load_x_whole(kt, 2, "kx", 4, nc.sync)
        # vx(2)/vx(3) reuse vx(0)/vx(1)'s slots: their DMAs self-delay on the
        # WAR dep instead of stalling the PE mid-block
        vxs[2] = load_x_whole(vt, 2, "vx", 2, nc.gpsimd)
        kxs[3] = load_x_whole(kt, 3, "kx", 4, nc.sync)
        vxs[3] = load_x_whole(vt, 3, "vx", 2, nc.gpsimd)

        kT = [kT_pool.tile([P, S], bf16, name=f"kT{i}", tag=f"kT{i}")
              for i in range(DST)]
        vA = [vA_pool.tile([P, NH, DK + 1], bf16, name=f"vA{i}", tag=f"vA{i}")
              for i in range(SKT)]

        # late-needed weights (queued behind the prologue stream)
        wo_sb = const.tile([P, NHP, 2, DH], bf16)

        # ---- PE p-state warm-up: tiny matmuls during the DMA wait ----
        # 3us of continuous PE execution ramps the clock 1.2->2.4GHz; these
        # ~180ns no-op matmuls burn the DMA-wait window so the first real
        # projection runs at full speed.
        for w in range(NWARM):
            wt = ps_big.tile([P, 2, DH], f32, tag="ps_big", name=f"warm{w}")
            nc.tensor.matmul(wt[0:NH, 0, 0:NH], lhsT=ones128, rhs=ones128,
                             start=True, stop=True)

        # ---- emitted-state tracking for the scheduler ----
        kg_done = set()    # (skb, ds)
        vg_done = set()    # (skb, j)
        qp_done = set()    # (nb, ds)
        qtiles = {nb: [None] * DST for nb in range(NB)}
        pairs = {nb: [None] * NHP for nb in range(NB)}

        # Each group is a list of micro-ops (~2 matmuls each, or one
        # eviction) so the scheduler can interleave them finely with the
        # attention stream.
        def kgroup(skb, ds):
            st = {}
            def mm(di0):
                def fn():
                    if di0 == 0:
                        st["ps"] = ps_mm.tile([P, DH], f32, tag="ps_mm",
                                              name=f"psk{skb}_{ds}")
                    for di in range(di0, di0 + 2):
                        nc.tensor.matmul(
                            st["ps"], lhsT=wk_sb[:, ds, di, :],
                            rhs=kxs[skb][:, di, :],
                            start=(di == 0), stop=(di == DIT - 1))
                return fn
            def ev():
                nc.vector.tensor_scalar_add(
                    kT[ds][:, skb * SQB:(skb + 1) * SQB], st["ps"],
                    bkt_sb[:, ds:ds + 1])
                kg_done.add((skb, ds))
            return [mm(0), mm(2), mm(4), mm(6), ev]

        def vgroup(skb, j):
            st = {}
            skt = skb * (SQB // P) + j
            def mm(di0):
                def fn():
                    if di0 == 0:
                        st["ps"] = ps_mm.tile([P, DH], f32, tag="ps_mm",
                                              name=f"psv{skb}_{j}")
                    for di in range(di0, di0 + 2):
                        nc.tensor.matmul(
                            st["ps"],
                            lhsT=vxs[skb][:, di, j * P:(j + 1) * P],
                            rhs=wv_sb[:, di, :],
                            start=(di == 0), stop=(di == DIT - 1))
                return fn
            def ev():
                va = vA[skt]
                nc.vector.tensor_copy(va[:, :, DK], ones128)
                nc.vector.tensor_tensor(
                    va[:, :, 0:DK], st["ps"].rearrange("p (h x) -> p h x", x=DK),
                    bv_sb.rearrange("p (h x) -> p h x", x=DK), ADD)
                vg_done.add((skb, j))
            return [mm(0), mm(2), mm(4), mm(6), ev]

        def qload(nb):
            def fn():
                t = xin_pool.tile([P, DIT, SQB], bf16, tag="xin", bufs=2,
                                  name=f"qx{nb}")
                nc.sync.dma_start(out=t,
                                  in_=qt[:, nb].rearrange("d p q -> p d q"))
                qx[nb] = t
            return [fn]

        def qgroup(nb, ds):
            st = {}
            def mm(di0):
                def fn():
                    if di0 == 0:
                        st["ps"] = ps_mm.tile([P, DH], f32, tag="ps_mm",
                                              name=f"psq{nb}_{ds}")
                    for di in range(di0, di0 + 2):
                        nc.tensor.matmul(
                            st["ps"], lhsT=wq_sb[:, ds, di, :],
                            rhs=qx[nb][:, di, :],
                            start=(di == 0), stop=(di == DIT - 1))
                return fn
            def ev():
                qtile = qT_pool.tile([P, SQB], bf16, tag="qT", name=f"qT{nb}_{ds}")
                nc.vector.tensor_scalar_add(qtile, st["ps"], bqt_sb[:, ds:ds + 1])
                qtiles[nb][ds] = qtile
                qp_done.add((nb, ds))
            return [mm(0), mm(2), mm(4), mm(6), ev]

        qT_pool = ctx.enter_context(tc.tile_pool(name="qT", bufs=8))
        probs_pool = ctx.enter_context(tc.tile_pool(name="probs", bufs=10))
        raw_pool = ctx.enter_context(tc.tile_pool(name="raw", bufs=6))
        pair_pool = ctx.enter_context(tc.tile_pool(name="pair", bufs=8))
        ostg_pool = ctx.enter_context(tc.tile_pool(name="ostg", bufs=2))
        bc_pool = ctx.enter_context(tc.tile_pool(name="bc", bufs=2))
        ob_pool = ctx.enter_context(tc.tile_pool(name="ob", bufs=4))

        def norm_pair(nb, hp, raw_e, raw_o):
            """DMA each raw tile's softmax-sums row straight to the DRAM
            bounce buffer, partition-broadcast it, reciprocal on the
            broadcast tile (base partition 0 - custom-DVE op constraint),
            DVE normalize into the [128,512] bf16 pair tile."""
            def fn():
                nc.sync.dma_start(out=scr[nb, 2 * hp:2 * hp + 1, :],
                                  in_=raw_e[DK:DK + 1, :])
                nc.sync.dma_start(out=scr[nb, 2 * hp + 1:2 * hp + 2, :],
                                  in_=raw_o[DK:DK + 1, :])
            def fn2():
                pair = pair_pool.tile([P, SQB], bf16, tag="pair",
                                      name=f"pair{nb}_{hp}")
                pairs[nb][hp] = pair
                bce = bc_pool.tile([DK, SQB], f32, tag="bc", name=f"bce{nb}_{hp}")
                nc.sync.dma_start(
                    out=bce, in_=scr[nb, 2 * hp, :].partition_broadcast(DK))
                nc.vector.reciprocal_approx_fast(bce, bce)
                nc.vector.tensor_mul(pair[0:DK, :], raw_e[0:DK, :], bce)
                bco = bc_pool.tile([DK, SQB], f32, tag="bc", name=f"bco{nb}_{hp}")
                nc.sync.dma_start(
                    out=bco, in_=scr[nb, 2 * hp + 1, :].partition_broadcast(DK))
                nc.vector.reciprocal_approx_fast(bco, bco)
                ostg = ostg_pool.tile([DK, SQB], bf16, tag="ostg",
                                      name=f"ostg{nb}_{hp}")
                nc.vector.tensor_mul(ostg, raw_o[0:DK, :], bco)
                nc.sync.dma_start(out=pair[DK:P, :], in_=ostg)
            return [fn, fn2]

        def norm_pair_pe(nb, hp, raw_e, raw_o):
            """Tail-pair variant: PE-matmul partition broadcast of the sums
            rows into PSUM (the PE is idle here; fp32 matmul is fine at
            this size) - avoids two serial DRAM-bounce DMA hops."""
            def fn():
                pair = pair_pool.tile([P, SQB], bf16, tag="pair",
                                      name=f"pair{nb}_{hp}")
                pairs[nb][hp] = pair
                # ps_big is idle by now (last exp emitted); ps_mm's two slots
                # are held by the pre-opened tail out-proj groups
                bcp2 = ps_big.tile([P, 2, DH], f32, tag="ps_big",
                                   name=f"bcp{nb}_{hp}")
                bcp = bcp2[:, 0, :]
                nc.tensor.matmul(bcp[0:DK, :], lhsT=onescolf[DK:DK + 1, :],
                                 rhs=raw_e[DK:DK + 1, :], start=True, stop=True)
                nc.tensor.matmul(bcp[DK:P, :], lhsT=onescolf[DK:DK + 1, :],
                                 rhs=raw_o[DK:DK + 1, :], start=True, stop=True)
                nc.vector.reciprocal_approx_fast(bcp, bcp)
                nc.vector.tensor_mul(pair[0:DK, :], raw_e[0:DK, :],
                                     bcp[0:DK, :])
                ostg = ostg_pool.tile([DK, SQB], bf16, tag="ostg",
                                      name=f"ostg{nb}_{hp}")
                nc.vector.tensor_mul(ostg, raw_o[0:DK, :], bcp[DK:P, :])
                nc.sync.dma_start(out=pair[DK:P, :], in_=ostg)
            return [fn]

        def outproj(nb, sq, nb2):
            st = {}
            def mm(hp0):
                def fn():
                    if hp0 == 0:
                        st["ps"] = ps_mm.tile([P, DH], f32, tag="ps_mm",
                                              name=f"pso{nb}_{sq}_{nb2}")
                    for hp in range(hp0, hp0 + 2):
                        nc.tensor.matmul(
                            st["ps"], lhsT=pairs[nb][hp][:, sq * P:(sq + 1) * P],
                            rhs=wo_sb[:, hp, nb2, :],
                            start=(hp == 0), stop=(hp == NHP - 1))
                return fn
            def ev():
                ob = ob_pool.tile([P, DH], bf16, tag="ob",
                                  name=f"ob{nb}_{sq}_{nb2}")
                nc.vector.tensor_copy(ob, st["ps"])
                # block 3's output DMAs stay off the gpsimd queue so its slow
                # engine drain overlaps the tail instead of following it
                eng = nc.gpsimd if (sq + nb2) % 2 == 0 and nb < NB - 1 \
                    else nc.sync
                eng.dma_start(
                    out=out[nb * SQB + sq * P: nb * SQB + (sq + 1) * P,
                            nb2 * DH:(nb2 + 1) * DH],
                    in_=ob)
            return [mm(0), mm(2), ev]

        # ---- attention iteration bodies ----
        def it_of(i):
            nb, r = divmod(i, NHP * SKT)
            hp, sk = divmod(r, SKT)
            return nb, hp, sk

        ps_of = {}
        pr_of = {}
        pa_of = {}
        fillers = deque()

        def ready_sc(i):
            nb, hp, sk = it_of(i)
            return ((sk // 4, hp) in kg_done) and ((nb, hp) in qp_done)

        def pump():
            assert fillers, "filler deque empty while consumer not ready"
            fillers.popleft()()

        def emit_sc(i):
            nb, hp, sk = it_of(i)
            while not ready_sc(i):
                pump()
            ps = ps_big.tile([P, 2, DH], f32, tag="ps_big", name=f"sc{i}")
            q = qtiles[nb][hp]
            nc.tensor.matmul(
                ps[:, 0, :], lhsT=kT[hp][0:DK, sk * P:(sk + 1) * P],
                rhs=q[0:DK, :], start=True, stop=True)
            nc.tensor.matmul(
                ps[:, 1, :], lhsT=kT[hp][DK:P, sk * P:(sk + 1) * P],
                rhs=q[DK:P, :], start=True, stop=True)
            ps_of[i] = ps

        def emit_exp(i):
            ps = ps_of.pop(i)
            pr = probs_pool.tile([P, 2, DH], bf16, tag="probs", name=f"pr{i}")
            nc.scalar.activation(pr.rearrange("p a b -> p (a b)"),
                                 ps.rearrange("p a b -> p (a b)"),
                                 EXP, scale=0.125)
            pr_of[i] = pr

        def emit_pv(i):
            nb, hp, sk = it_of(i)
            while (sk // 4, sk % 4) not in vg_done:
                pump()
            if sk == 0:
                pa_e = ps_attn.tile([DK + 1, DH], f32, tag="ps_attn",
                                    name=f"pae{nb}_{hp}")
                pa_o = ps_attn.tile([DK + 1, DH], f32, tag="ps_attn",
                                    name=f"pao{nb}_{hp}")
                pa_of[(nb, hp)] = (pa_e, pa_o)
            pa_e, pa_o = pa_of[(nb, hp)]
            pr = pr_of.pop(i)
            nc.tensor.matmul(pa_e, lhsT=vA[sk][:, 2 * hp, :], rhs=pr[:, 0, :],
                             start=(sk == 0), stop=(sk == SKT - 1))
            nc.tensor.matmul(pa_o, lhsT=vA[sk][:, 2 * hp + 1, :], rhs=pr[:, 1, :],
                             start=(sk == 0), stop=(sk == SKT - 1))
            if sk == SKT - 1:
                # pair done: evict attn rows (+sums row), queue its norm
                raws = []
                for pa, h in ((pa_e, 2 * hp), (pa_o, 2 * hp + 1)):
                    raw = raw_pool.tile([DK + 1, SQB], f32, tag="raw",
                                        name=f"raw{nb}_{h}")
                    nc.vector.tensor_copy(raw, pa)
                    raws.append(raw)
                del pa_of[(nb, hp)]
                if nb == NB - 1 and hp == NHP - 1:
                    fillers.extend(norm_pair_pe(nb, hp, raws[0], raws[1]))
                else:
                    fillers.extend(norm_pair(nb, hp, raws[0], raws[1]))

        # ---- prologue head: only the two groups sc(0) needs run inline ----
        for op in kgroup(0, 0) + qgroup(0, 0):
            op()

        # block-0 filler schedule, ordered by first need (v-projections for
        # pv(0..15) first - emit_pv's pump forces any stragglers)
        for g in [
            vgroup(0, 0), vgroup(0, 1), vgroup(0, 2), vgroup(0, 3),
            kgroup(1, 0),
            vgroup(1, 0), vgroup(1, 1), vgroup(1, 2), vgroup(1, 3),
            kgroup(2, 0),
            vgroup(2, 0), vgroup(2, 1), vgroup(2, 2), vgroup(2, 3),
            kgroup(3, 0),
            vgroup(3, 0), vgroup(3, 1), vgroup(3, 2), vgroup(3, 3),
            qgroup(0, 1), kgroup(0, 1), kgroup(1, 1),
            kgroup(2, 1), kgroup(3, 1),
            qgroup(0, 2), kgroup(0, 2), kgroup(1, 2), kgroup(2, 2),
            kgroup(3, 2),
            qgroup(0, 3), kgroup(0, 3),
            qload(1),
            [lambda: nc.gpsimd.dma_start(out=wo_sb, in_=wo[:, :, :, :])],
            kgroup(1, 3), qgroup(1, 0), kgroup(2, 3), qgroup(1, 1),
            kgroup(3, 3), qgroup(1, 2), qgroup(1, 3),
        ]:
            fillers.extend(g)

        # ---- main software-pipelined loop (PV lagged LAG iterations, so
        # block-0 V-projection deadlines stretch and the PE never waits on
        # the exp stream) ----
        LAG = 8
        emit_sc(0)
        emit_sc(1)
        for i in range(T + LAG):
            j = i - LAG  # the PV (and block-boundary bookkeeping) iteration
            if j >= 1 and j % (NHP * SKT) == 0:
                # j's block boundary: queue prev block's out-proj + next q-proj
                # (after norm(nb-1, 3), which pv(j-1) just appended)
                nbj = j // (NHP * SKT)
                for sq in range(SQB // P):
                    for nb2 in range(2):
                        if nbj - 1 == NB - 1:
                            break
                        fillers.extend(outproj(nbj - 1, sq, nb2))
                if nbj + 1 < NB:
                    fillers.extend(qload(nbj + 1))
                    for ds in range(DST):
                        fillers.extend(qgroup(nbj + 1, ds))
            if j == T - 16:
                # open the first two block-3 out-proj psum groups while the
                # last pair is still streaming (pairs 3,0/3,1 are ready);
                # keeps the PE at speed through the final normalization
                op30, op31 = outproj(NB - 1, 0, 0), outproj(NB - 1, 0, 1)
                tail_groups = [op30, op31]
                fillers.append(op30[0])
                fillers.append(op31[0])
            if i < T:
                emit_exp(i)
            # pops and pv run on the PE while exp(i) streams on ACT; sc(i+2)
            # is emitted last so the PE reaches it right as exp(i)'s PSUM
            # read (the WAR it must wait on) completes - minimal PE idle.
            # Early iterations hold fillers back: their matmuls would stall
            # on prologue DMAs and queue-block the ready scores behind them.
            npop = 0 if i < 6 else (6 if i < 28 else (3 if i < 56 else 1))
            for _ in range(npop):
                if fillers:
                    fillers.popleft()()
            if j >= 0:
                emit_pv(j)
            if i + 2 < T:
                emit_sc(i + 2)

        # ---- tail: drain fillers (incl. norm(3,3)), then block 3 out-proj
        # with psum groups staggered so <=2 are open at once ----
        while fillers:
            fillers.popleft()()
        remaining = [outproj(NB - 1, sq, nb2)
                     for sq in range(SQB // P) for nb2 in range(2)][2:]
        open_q = deque(tail_groups)
        for g in remaining:
            # close-then-open keeps <=2 groups live (ps_mm has 2 slots)
            h = open_q.popleft()
            h[1]()                      # close: hp 2-3 matmuls
            h[2]()                      # evict + output DMA
            g[0]()                      # open: hp 0-1 matmuls
            open_q.append(g)
        while open_q:
            h = open_q.popleft()
            h[1]()
            h[2]()

    nc.compile()
    return nc


_NC_CACHE = {}


def _get_nc():
    if "nc" not in _NC_CACHE:
        _NC_CACHE["nc"] = build()
    return _NC_CACHE["nc"]


def _tile_xt(x):
    # [S, DM] -> transpose -> [DIT, NB, P, SQB] with each [P, SQB] contiguous
    xt = np.ascontiguousarray(x.T)                      # [DM, S]
    return np.ascontiguousarray(
        xt.reshape(DIT, P, NB, SQB).transpose(0, 2, 1, 3)).astype(BF)


def _wcol_dsmajor(W, cs):
    # [DM, DH-slice] -> [DST, P, DIT, P] (ds-major stationary layout)
    return np.ascontiguousarray(
        W[:, cs].reshape(DIT, P, DST, P).transpose(2, 1, 0, 3)).astype(BF)


def _shard_inputs(Q, K, V, Wq, bq, Wk, bk, Wv, bv, Wo, bo):
    in_maps = []
    qkvT = {}
    for b in range(4):
        qkvT[b] = (_tile_xt(Q[b]), _tile_xt(K[b]), _tile_xt(V[b]))
    halves = []
    for h in range(2):
        cs = slice(h * DH, (h + 1) * DH)
        halves.append(dict(
            wq=_wcol_dsmajor(Wq, cs),
            wk=_wcol_dsmajor(Wk, cs),
            wv=np.ascontiguousarray(
                Wv[:, cs].reshape(DIT, P, DH).transpose(1, 0, 2)).astype(BF),
            wo=np.ascontiguousarray(
                Wo[cs, :].reshape(NHP, P, 2, DH).transpose(1, 0, 2, 3)).astype(BF),
            bqt=np.ascontiguousarray(
                bq[cs].reshape(DST, P).T).astype(np.float32),
            bkt=np.ascontiguousarray(
                bk[cs].reshape(DST, P).T).astype(np.float32),
            bv=bv[cs].reshape(1, DH).astype(np.float32),
        ))
    for c in range(8):
        b, h = c // 2, c % 2
        qT, kT_, vT = qkvT[b]
        m = dict(qt=qT, kt=kT_, vt=vT)
        m.update(halves[h])
        in_maps.append(m)
    return in_maps


TRACE = False
LAST_RESULT = None


def kernel(**inputs):
    global LAST_RESULT
    inputs = {k: np.asarray(v, dtype=np.float32) for k, v in inputs.items()}
    nc = _get_nc()
    in_maps = _shard_inputs(
        inputs["Q"], inputs["K"], inputs["V"],
        inputs["Wq"], inputs["bq"], inputs["Wk"], inputs["bk"],
        inputs["Wv"], inputs["bv"], inputs["Wo"], inputs["bo"])
    r = run_bass_kernel_spmd(nc, in_maps, core_ids=list(range(8)), trace=TRACE)
    LAST_RESULT = r
    outs = [np.asarray(r.results[c]["out"], dtype=np.float32) for c in range(8)]
    full = np.stack([outs[2 * b] + outs[2 * b + 1] for b in range(4)], axis=0)
    return full + inputs["bo"].reshape(1, 1, DM)
